# revision 26
# baseline (speedup 1.0000x reference)
"""Trainium2 Bass kernel: 3-layer LSTM EEG classifier (B=64, C=64, T=1000, H=512, NC=5).

Sharding: data-parallel over batch -> 8 cores x 8 samples, weights replicated.

Per-core schedule (per LSTM layer):
  1. PROJ: xg = W_ih @ x_seq + b as a big throughput matmul (N=512 free dim),
     written to a DRAM scratch buffer in a gate-permuted "chunk" layout.
  2. TIME LOOP: 1000 sequential steps; per step 64 weight-stationary matmuls
     (16 gate chunks x 4 K-tiles, N=8) accumulate gates^T in 4 per-quarter
     PSUM banks, then per-quarter elementwise (sigmoid/tanh/cell update) on
     DVE+ACT overlapped with the next quarter's matmuls.
All matmul operands bf16 (fp32 accumulate); gates/cell state fp32.

Layouts (per core, P=128 partitions):
  gates^T tile [P, 128]: col = tau*32 + j*8 + b, where tau in {i,f,o,g} (gate
  type, host-permuted row order), j = hidden quarter (u = j*128 + p), b = batch.
  h^T / c^T tiles [P, 32]: col = j*8 + b.  h_seq keeps T+1 slots in SBUF (bf16).
"""

import numpy as np
import ml_dtypes

P = 128
B, C, T_FULL, H, L, NCLS = 64, 64, 1000, 512, 3, 5
G = 4 * H            # 2048 gate rows
KH = H // P          # 4 K-tiles over hidden
NCH = G // P         # 16 gate chunks
NCORES = 8
BL = B // NCORES     # 8 samples per core
U_DEF = 20           # time-loop unroll (body steps per hw loop iteration)

BF16 = ml_dtypes.bfloat16


def build_program(T=T_FULL, U=U_DEF, variant="full",
                  SWEEP_ORDER=((0, 0), (0, 1), (1, 0), (1, 1))):
    """Build the Bass program (single NeuronCore, run SPMD on 8 cores).

    variant: "full" | "mmonly" (skip elementwise, h via copy) |
             "ewonly" (skip matmuls) | "nohchain" (h written to scratch;
             breaks recurrence dep to measure pipelined throughput).
    Non-"full" variants produce wrong numerics; timing only.
    """
    import concourse.bass as bass
    import concourse.mybir as mybir
    import concourse.tile as tile
    from concourse import bacc
    from concourse.bass import ds

    assert T % U == 0
    f32 = mybir.dt.float32
    bf16 = mybir.dt.bfloat16
    AF = mybir.ActivationFunctionType

    nc = bacc.Bacc("TRN2", target_bir_lowering=False, debug=False)

    # ---------------- I/O ----------------
    xT = nc.dram_tensor("xT", [P, T * BL], bf16, kind="ExternalInput")
    wih0 = nc.dram_tensor("wih0", [P, G], bf16, kind="ExternalInput")
    wih12 = nc.dram_tensor("wih12", [2, KH, P, G], bf16, kind="ExternalInput")
    whh = nc.dram_tensor("whh", [L, KH, P, G], bf16, kind="ExternalInput")
    bias = nc.dram_tensor("bias", [L, P, NCH], f32, kind="ExternalInput")
    wfc = nc.dram_tensor("wfc", [KH, P, NCLS], bf16, kind="ExternalInput")
    bfc = nc.dram_tensor("bfc", [NCLS, 1], f32, kind="ExternalInput")
    out = nc.dram_tensor("out", [BL, NCLS], f32, kind="ExternalOutput")
    xg_d = [
        nc.dram_tensor(f"xg{i}", [P, T * P], f32, kind="Internal") for i in range(2)
    ]

    # ---------------- persistent SBUF ----------------
    whh_sb = nc.alloc_sbuf_tensor("whh_sb", [P, L * KH * G], bf16).ap()
    wih_sb = nc.alloc_sbuf_tensor("wih_sb", [P, 2 * KH * G], bf16).ap()
    wih0_sb = nc.alloc_sbuf_tensor("wih0_sb", [P, G], bf16).ap()
    x_sb = nc.alloc_sbuf_tensor("x_sb", [P, T * BL], bf16).ap()
    hseq = nc.alloc_sbuf_tensor("hseq", [P, (T + 1) * 4 * BL], bf16).ap()
    csb = nc.alloc_sbuf_tensor("csb", [P, 4 * BL], f32).ap()
    bias_sb = nc.alloc_sbuf_tensor("bias_sb", [P, L * NCH], f32).ap()
    wfc_sb = nc.alloc_sbuf_tensor("wfc_sb", [P, KH * NCLS], bf16).ap()
    bfc_sb = nc.alloc_sbuf_tensor("bfc_sb", [NCLS, 1], f32).ap()
    # static staging for the time loop: all per-step APs stay register-free
    # (dynamic-offset APs burn a per-engine register per instruction per loop
    # body, budget ~12 — only the few staging DMAs below use dynamic offsets)
    stage_h = nc.alloc_sbuf_tensor("stage_h", [P, U * 4 * BL], bf16).ap()
    stage_xg = nc.alloc_sbuf_tensor("stage_xg", [P, U * P], f32).ap()
    hscr = nc.alloc_sbuf_tensor("hscr", [P, 4 * BL], bf16).ap()  # nohchain sink

    hv = hseq.rearrange("p (t x) -> p t x", x=4 * BL)  # [P, T+1, 32]

    chunks = [(t0, min(64, T - t0)) for t0 in range(0, T, 64)]

    # xg column-block permutation: chunk n (= tau*4 + j) lands at block
    # chi*8 + tau*2 + jj so each j-half's gates are CONTIGUOUS 64 columns
    # (tau-major within half) — keeps every elementwise op a flat slice.
    CPRM = [(n % 4 // 2) * 8 + (n // 4) * 2 + (n % 4 % 2) for n in range(NCH)]

    with tile.TileContext(nc) as tc:
        with (
            tc.tile_pool(name="gp", bufs=3) as g_pool,
            tc.tile_pool(name="tmpp", bufs=4) as tmp_pool,
            tc.tile_pool(name="epp", bufs=3) as ep_pool,
            tc.tile_pool(name="psA", bufs=2, space="PSUM") as psA,
        ):
            # ---- load weights/inputs into SBUF ----
            for l in range(L):
                for k in range(KH):
                    o = (l * KH + k) * G
                    nc.sync.dma_start(whh_sb[:, o : o + G], whh.ap()[l, k])
            for l in range(2):
                for k in range(KH):
                    o = (l * KH + k) * G
                    nc.sync.dma_start(wih_sb[:, o : o + G], wih12.ap()[l, k])
            nc.sync.dma_start(wih0_sb, wih0.ap())
            nc.sync.dma_start(x_sb, xT.ap())
            for l in range(L):
                nc.sync.dma_start(bias_sb[:, l * NCH : (l + 1) * NCH], bias.ap()[l])
            for k in range(KH):
                nc.sync.dma_start(wfc_sb[:, k * NCLS : (k + 1) * NCLS], wfc.ap()[k])
            nc.sync.dma_start(bfc_sb, bfc.ap())
            nc.vector.memset(hseq[:, 0 : 4 * BL], 0.0)  # h_{-1} = 0 slot

            for l in range(L):
                xg = xg_d[l % 2].ap()  # [P, T*P]
                xgv = xg.rearrange("p (t m) -> p t m", m=P)
                kt = 1 if l == 0 else KH

                # ---------- PROJ: xg = W_ih @ x + bias ----------
                for t0, tcnt in chunks:
                    ncols = tcnt * BL
                    for n in range(NCH):
                        ps = psA.tile([P, 512], f32, tag="bkA")
                        for k in range(kt):
                            if l == 0:
                                lhsT = wih0_sb[:, n * P : (n + 1) * P]
                                rhs = x_sb[:, t0 * BL : (t0 + tcnt) * BL]
                            else:
                                o = ((l - 1) * KH + k) * G
                                lhsT = wih_sb[:, o + n * P : o + (n + 1) * P]
                                rhs = hv[:, t0 + 1 : t0 + 1 + tcnt, k * BL : (k + 1) * BL]
                            nc.tensor.matmul(
                                ps[:, :ncols], lhsT=lhsT, rhs=rhs,
                                start=(k == 0), stop=(k == kt - 1),
                            )
                        ep = ep_pool.tile([P, 512], f32, tag="ep")
                        nc.vector.tensor_scalar_add(
                            ep[:, :ncols], ps[:, :ncols],
                            bias_sb[:, l * NCH + n : l * NCH + n + 1],
                        )
                        nc.sync.dma_start(
                            xgv[:, t0 : t0 + tcnt, CPRM[n] * BL : (CPRM[n] + 1) * BL],
                            ep[:, :ncols].rearrange("p (t b) -> p t b", b=BL),
                        )

                # ---------- TIME LOOP ----------
                nc.vector.memset(csb, 0.0)
                # h_{-1} = 0: slot U-1 of stage_h is what step u=0 reads
                nc.vector.memset(stage_h[:, (U - 1) * 4 * BL : U * 4 * BL], 0.0)
                nit = T // U
                XCH = 4  # xg staging DMA chunks per body
                assert U % XCH == 0
                with tc.For_i(
                    0, nit, 1, hint_engines=(mybir.EngineType.PE,)
                ) as it:
                    for c in range(XCH):
                        w = (U // XCH) * P
                        nc.sync.dma_start(
                            stage_xg[:, c * w : (c + 1) * w],
                            xg[:, ds(it * (U * P) + c * w, w)],
                        )
                    for u in range(U):
                        xg_t = stage_xg[:, u * P : (u + 1) * P]
                        g_sb = g_pool.tile([P, P], f32, tag="g")
                        rd = ((u - 1) % U) * (4 * BL)  # h_{t-1} staging slot
                        wr = u * (4 * BL)              # h_t staging slot
                        wo = l * KH * G
                        # One psum BANK per j-half; all 4 K-tiles accumulate
                        # into it. Exactly one start=True per bank per step
                        # (clears the whole bank's has_written bits); every
                        # chunk's first write then overwrites (its bits are
                        # clear) and later k's accumulate — saving a second
                        # DVE add. k-pair sweeps are still emitted kappa-outer
                        # so the next step's k01 sweeps depend only on h q0/q1.
                        pst = [None, None]

                        def mm_sweep(ki, ci):
                            if ki == 0:
                                pst[ci] = psA.tile(
                                    [P, 8 * BL], f32,
                                    tag=("bkC", "bkD")[ci], name=f"ps{ci}",
                                )
                            ps = pst[ci]
                            if variant == "ewonly":
                                if ki == 0:
                                    nc.vector.memset(ps, 0.01)
                                return
                            for jj in range(2):
                                j = ci * 2 + jj
                                for tau in range(4):
                                    nch = tau * 4 + j
                                    for k in range(2 * ki, 2 * ki + 2):
                                        nc.tensor.matmul(
                                            ps[:, (tau * 2 + jj) * BL : (tau * 2 + jj + 1) * BL],
                                            lhsT=whh_sb[
                                                :,
                                                wo + k * G + nch * P : wo + k * G + (nch + 1) * P,
                                            ],
                                            rhs=stage_h[:, rd + k * BL : rd + (k + 1) * BL],
                                            start=(ki == 0 and jj == 0 and tau == 0 and k == 0),
                                            stop=(k == KH - 1),
                                            skip_group_check=True,
                                        )

                        def ew_half(ci):
                            # contiguous half block: cols [ci*64, ci*64+64),
                            # tau-major: i [0:16), f [16:32), o [32:48), g [48:64)
                            HB = 8 * BL
                            gh = g_sb[:, ci * HB : (ci + 1) * HB]
                            xh = xg_t[:, ci * HB : (ci + 1) * HB]
                            nc.vector.tensor_add(gh, pst[ci], xh)
                            nc.scalar.activation(
                                gh[:, 0 : 6 * BL], gh[:, 0 : 6 * BL], AF.Sigmoid)
                            nc.scalar.activation(
                                gh[:, 6 * BL : 8 * BL], gh[:, 6 * BL : 8 * BL], AF.Tanh)
                            ig = tmp_pool.tile([P, 2 * BL], f32, tag="ig")
                            nc.vector.tensor_mul(
                                ig, gh[:, 0 : 2 * BL], gh[:, 6 * BL : 8 * BL])
                            cq = csb[:, 2 * ci * BL : (2 * ci + 2) * BL]
                            nc.vector.tensor_mul(cq, gh[:, 2 * BL : 4 * BL], cq)
                            nc.vector.tensor_add(cq, cq, ig)
                            tc_ = tmp_pool.tile([P, 2 * BL], f32, tag="tc")
                            nc.scalar.activation(tc_, cq, AF.Tanh)
                            hdst = hscr if variant == "nohchain" else stage_h
                            hoff = (0 if variant == "nohchain" else wr) + 2 * ci * BL
                            nc.vector.tensor_mul(
                                hdst[:, hoff : hoff + 2 * BL], gh[:, 4 * BL : 6 * BL], tc_)

                        done = set()
                        emitted = set()
                        for ki, ci in SWEEP_ORDER:
                            mm_sweep(ki, ci)
                            done.add((ki, ci))
                            if variant == "mmonly":
                                continue
                            for ch in range(2):
                                if ch not in emitted and {(0, ch), (1, ch)} <= done:
                                    ew_half(ch)
                                    emitted.add(ch)
                        if variant == "mmonly":
                            for ci in range(2):
                                nc.vector.tensor_copy(
                                    stage_h[:, wr + 2 * ci * BL : wr + (2 * ci + 2) * BL],
                                    pst[ci][:, 0 : 2 * BL],
                                )
                            continue
                    # persist this body's h_t slots into hseq[it*U+1 .. it*U+U]
                    nc.sync.dma_start(
                        hseq[:, ds(it * (U * 4 * BL) + 4 * BL, U * 4 * BL)], stage_h
                    )

            # ---------- FC head ----------
            psf = psA.tile([NCLS, BL], f32, tag="bkB")
            for k in range(KH):
                nc.tensor.matmul(
                    psf,
                    lhsT=wfc_sb[:, k * NCLS : (k + 1) * NCLS],
                    rhs=hv[:, T, k * BL : (k + 1) * BL],
                    start=(k == 0), stop=(k == KH - 1),
                )
            osb = tmp_pool.tile([NCLS, BL], f32, tag="osb")
            nc.vector.tensor_scalar_add(osb, psf, bfc_sb)
            nc.sync.dma_start(out.ap().rearrange("b c -> c b"), osb)

    nc.compile()
    return nc


# ---------------- host-side input prep ----------------

_GATE_PERM = np.concatenate(
    [np.arange(0, H), np.arange(H, 2 * H), np.arange(3 * H, 4 * H), np.arange(2 * H, 3 * H)]
)  # reorder gate blocks [i, f, g, o] -> [i, f, o, g]


def prep_weights(W_ih0, W_ih_rest, W_hh, b_ih, b_hh, W_fc, b_fc, T=T_FULL):
    """Host-side: permute/transpose/tile/cast weights into kernel input layout."""
    W_ih0 = np.asarray(W_ih0, np.float32)[_GATE_PERM]          # [G, C]
    wih0 = np.zeros((P, G), np.float32)
    wih0[:C] = W_ih0.T                                          # K-padded lhsT
    wih12 = np.stack(
        [np.asarray(W_ih_rest[i], np.float32)[_GATE_PERM].T.reshape(KH, P, G) for i in range(L - 1)]
    )                                                           # [2, KH, P, G]
    whh = np.stack(
        [np.asarray(W_hh[i], np.float32)[_GATE_PERM].T.reshape(KH, P, G) for i in range(L)]
    )                                                           # [L, KH, P, G]
    bsum = (np.asarray(b_ih, np.float32) + np.asarray(b_hh, np.float32))[:, _GATE_PERM]
    bias = np.ascontiguousarray(bsum.reshape(L, NCH, P).transpose(0, 2, 1))  # [L, P, NCH]
    wfc = np.asarray(W_fc, np.float32).T.reshape(KH, P, NCLS)   # [KH, P, NCLS]
    bfc = np.asarray(b_fc, np.float32).reshape(NCLS, 1)
    return {
        "wih0": wih0.astype(BF16),
        "wih12": wih12.astype(BF16),
        "whh": whh.astype(BF16),
        "bias": np.ascontiguousarray(bias, np.float32),
        "wfc": wfc.astype(BF16),
        "bfc": bfc,
    }


def prep_x_core(x_core, T=T_FULL):
    """x_core [BL, C, T] -> padded xT [P, T*BL] bf16 (col = t*BL + b)."""
    xt = np.zeros((P, T * BL), np.float32)
    xt[:C] = np.asarray(x_core, np.float32).transpose(1, 2, 0).reshape(C, T * BL)
    return xt.astype(BF16)


_CACHE = {}


def kernel(x, W_ih0, W_ih_rest, W_hh, b_ih, b_hh, W_fc, b_fc):
    from concourse.bass_utils import run_bass_kernel_spmd

    x = np.asarray(x, np.float32)
    wts = prep_weights(W_ih0, W_ih_rest, W_hh, b_ih, b_hh, W_fc, b_fc)
    in_maps = []
    for c in range(NCORES):
        m = dict(wts)
        m["xT"] = prep_x_core(x[c * BL : (c + 1) * BL])
        in_maps.append(m)

    if "nc" not in _CACHE:
        _CACHE["nc"] = build_program()
    res = run_bass_kernel_spmd(_CACHE["nc"], in_maps, core_ids=list(range(NCORES)))
    return np.concatenate([r["out"] for r in res.results], axis=0).astype(np.float32)


if __name__ == "__main__":
    rng = np.random.default_rng(0)
    s = 1.0 / np.sqrt(H)
    ins = dict(
        x=rng.standard_normal((B, C, T_FULL), dtype=np.float32),
        W_ih0=rng.uniform(-s, s, (G, C)).astype(np.float32),
        W_ih_rest=rng.uniform(-s, s, (L - 1, G, H)).astype(np.float32),
        W_hh=rng.uniform(-s, s, (L, G, H)).astype(np.float32),
        b_ih=rng.uniform(-s, s, (L, G)).astype(np.float32),
        b_hh=rng.uniform(-s, s, (L, G)).astype(np.float32),
        W_fc=rng.uniform(-s, s, (NCLS, H)).astype(np.float32),
        b_fc=rng.uniform(-s, s, (NCLS,)).astype(np.float32),
    )
    out = kernel(**ins)
    print(out.shape, out.dtype, np.abs(out).max())


# revision 30
# speedup vs baseline: 177.0263x; 177.0263x over previous
"""Trainium2 Bass kernel: 3-layer LSTM EEG classifier (B=64, C=64, T=1000, H=512, NC=5).

Sharding: data-parallel over batch -> 8 cores x 8 samples, weights replicated.

Per-core schedule (per LSTM layer):
  1. PROJ: xg = W_ih @ x_seq + b as a big throughput matmul (N=512 free dim),
     written to a DRAM scratch buffer in a gate-permuted "chunk" layout.
  2. TIME LOOP: 1000 sequential steps; per step 64 weight-stationary matmuls
     (16 gate chunks x 4 K-tiles, N=8) accumulate gates^T in 4 per-quarter
     PSUM banks, then per-quarter elementwise (sigmoid/tanh/cell update) on
     DVE+ACT overlapped with the next quarter's matmuls.
All matmul operands bf16 (fp32 accumulate); gates/cell state fp32.

Layouts (per core, P=128 partitions):
  gates^T tile [P, 128]: col = tau*32 + j*8 + b, where tau in {i,f,o,g} (gate
  type, host-permuted row order), j = hidden quarter (u = j*128 + p), b = batch.
  h^T / c^T tiles [P, 32]: col = j*8 + b.  h_seq keeps T+1 slots in SBUF (bf16).
"""

import numpy as np
import ml_dtypes

P = 128
B, C, T_FULL, H, L, NCLS = 64, 64, 1000, 512, 3, 5
G = 4 * H            # 2048 gate rows
KH = H // P          # 4 K-tiles over hidden
NCH = G // P         # 16 gate chunks
NCORES = 8
BL = B // NCORES     # 8 samples per core
U_DEF = 20           # time-loop unroll (body steps per hw loop iteration)

BF16 = ml_dtypes.bfloat16


def build_program(T=T_FULL, U=U_DEF, variant="full",
                  SWEEP_ORDER=((0, 0), (0, 1), (1, 0), (1, 1))):
    """Build the Bass program (single NeuronCore, run SPMD on 8 cores).

    variant: "full" | "mmonly" (skip elementwise, h via copy) |
             "ewonly" (skip matmuls) | "nohchain" (h written to scratch;
             breaks recurrence dep to measure pipelined throughput).
    Non-"full" variants produce wrong numerics; timing only.
    """
    import concourse.bass as bass
    import concourse.mybir as mybir
    import concourse.tile as tile
    from concourse import bacc
    from concourse.bass import ds

    assert T % U == 0
    f32 = mybir.dt.float32
    bf16 = mybir.dt.bfloat16
    AF = mybir.ActivationFunctionType

    nc = bacc.Bacc("TRN2", target_bir_lowering=False, debug=False)

    # ---------------- I/O ----------------
    xT = nc.dram_tensor("xT", [P, T * BL], bf16, kind="ExternalInput")
    wih0 = nc.dram_tensor("wih0", [P, G], bf16, kind="ExternalInput")
    wih12 = nc.dram_tensor("wih12", [2, KH, P, G], bf16, kind="ExternalInput")
    whh = nc.dram_tensor("whh", [L, KH, P, G], bf16, kind="ExternalInput")
    bias = nc.dram_tensor("bias", [L, P, NCH], f32, kind="ExternalInput")
    wfc = nc.dram_tensor("wfc", [KH, P, NCLS], bf16, kind="ExternalInput")
    bfc = nc.dram_tensor("bfc", [NCLS, 1], f32, kind="ExternalInput")
    out = nc.dram_tensor("out", [BL, NCLS], f32, kind="ExternalOutput")
    xg_d = [
        nc.dram_tensor(f"xg{i}", [P, T * P], f32, kind="Internal") for i in range(2)
    ]

    # ---------------- persistent SBUF ----------------
    whh_sb = nc.alloc_sbuf_tensor("whh_sb", [P, L * KH * G], bf16).ap()
    wih_sb = nc.alloc_sbuf_tensor("wih_sb", [P, 2 * KH * G], bf16).ap()
    wih0_sb = nc.alloc_sbuf_tensor("wih0_sb", [P, G], bf16).ap()
    x_sb = nc.alloc_sbuf_tensor("x_sb", [P, T * BL], bf16).ap()
    hseq = nc.alloc_sbuf_tensor("hseq", [P, (T + 1) * 4 * BL], bf16).ap()
    csb = nc.alloc_sbuf_tensor("csb", [P, 4 * BL], f32).ap()
    bias_sb = nc.alloc_sbuf_tensor("bias_sb", [P, L * NCH], f32).ap()
    wfc_sb = nc.alloc_sbuf_tensor("wfc_sb", [P, KH * NCLS], bf16).ap()
    bfc_sb = nc.alloc_sbuf_tensor("bfc_sb", [NCLS, 1], f32).ap()
    # static staging for the time loop: all per-step APs stay register-free
    # (dynamic-offset APs burn a per-engine register per instruction per loop
    # body, budget ~12 — only the few staging DMAs below use dynamic offsets)
    stage_h = nc.alloc_sbuf_tensor("stage_h", [P, U * 4 * BL], bf16).ap()
    stage_xg = nc.alloc_sbuf_tensor("stage_xg", [P, U * P], f32).ap()
    hscr = nc.alloc_sbuf_tensor("hscr", [P, 4 * BL], bf16).ap()  # nohchain sink

    hv = hseq.rearrange("p (t x) -> p t x", x=4 * BL)  # [P, T+1, 32]

    chunks = [(t0, min(64, T - t0)) for t0 in range(0, T, 64)]

    # xg column-block permutation: chunk n (= tau*4 + j) lands at block
    # chi*8 + tau*2 + jj so each j-half's gates are CONTIGUOUS 64 columns
    # (tau-major within half) — keeps every elementwise op a flat slice.
    CPRM = [(n % 4 // 2) * 8 + (n // 4) * 2 + (n % 4 % 2) for n in range(NCH)]

    with tile.TileContext(nc) as tc:
        with (
            tc.tile_pool(name="gp", bufs=3) as g_pool,
            tc.tile_pool(name="tmpp", bufs=4) as tmp_pool,
            tc.tile_pool(name="epp", bufs=3) as ep_pool,
            tc.tile_pool(name="psA", bufs=2, space="PSUM") as psA,
        ):
            # ---- load weights/inputs into SBUF ----
            for l in range(L):
                for k in range(KH):
                    o = (l * KH + k) * G
                    nc.sync.dma_start(whh_sb[:, o : o + G], whh.ap()[l, k])
            for l in range(2):
                for k in range(KH):
                    o = (l * KH + k) * G
                    nc.sync.dma_start(wih_sb[:, o : o + G], wih12.ap()[l, k])
            nc.sync.dma_start(wih0_sb, wih0.ap())
            nc.sync.dma_start(x_sb, xT.ap())
            for l in range(L):
                nc.sync.dma_start(bias_sb[:, l * NCH : (l + 1) * NCH], bias.ap()[l])
            for k in range(KH):
                nc.sync.dma_start(wfc_sb[:, k * NCLS : (k + 1) * NCLS], wfc.ap()[k])
            nc.sync.dma_start(bfc_sb, bfc.ap())
            nc.vector.memset(hseq[:, 0 : 4 * BL], 0.0)  # h_{-1} = 0 slot

            for l in range(L):
                xg = xg_d[l % 2].ap()  # [P, T*P]
                xgv = xg.rearrange("p (t m) -> p t m", m=P)
                kt = 1 if l == 0 else KH

                # ---------- PROJ: xg = W_ih @ x + bias ----------
                for t0, tcnt in chunks:
                    ncols = tcnt * BL
                    for n in range(NCH):
                        ps = psA.tile([P, 512], f32, tag="bkA")
                        for k in range(kt):
                            if l == 0:
                                lhsT = wih0_sb[:, n * P : (n + 1) * P]
                                rhs = x_sb[:, t0 * BL : (t0 + tcnt) * BL]
                            else:
                                o = ((l - 1) * KH + k) * G
                                lhsT = wih_sb[:, o + n * P : o + (n + 1) * P]
                                rhs = hv[:, t0 + 1 : t0 + 1 + tcnt, k * BL : (k + 1) * BL]
                            nc.tensor.matmul(
                                ps[:, :ncols], lhsT=lhsT, rhs=rhs,
                                start=(k == 0), stop=(k == kt - 1),
                            )
                        ep = ep_pool.tile([P, 512], f32, tag="ep")
                        nc.vector.tensor_scalar_add(
                            ep[:, :ncols], ps[:, :ncols],
                            bias_sb[:, l * NCH + n : l * NCH + n + 1],
                        )
                        nc.sync.dma_start(
                            xgv[:, t0 : t0 + tcnt, CPRM[n] * BL : (CPRM[n] + 1) * BL],
                            ep[:, :ncols].rearrange("p (t b) -> p t b", b=BL),
                        )

                # ---------- TIME LOOP ----------
                nc.vector.memset(csb, 0.0)
                # h_{-1} = 0: slot U-1 of stage_h is what step u=0 reads
                nc.vector.memset(stage_h[:, (U - 1) * 4 * BL : U * 4 * BL], 0.0)
                nit = T // U
                XCH = 4  # xg staging DMA chunks per body
                assert U % XCH == 0
                with tc.For_i(
                    0, nit, 1, hint_engines=(mybir.EngineType.PE,)
                ) as it:
                    for c in range(XCH):
                        w = (U // XCH) * P
                        nc.sync.dma_start(
                            stage_xg[:, c * w : (c + 1) * w],
                            xg[:, ds(it * (U * P) + c * w, w)],
                        )
                    for u in range(U):
                        xg_t = stage_xg[:, u * P : (u + 1) * P]
                        g_sb = g_pool.tile([P, P], f32, tag="g")
                        rd = ((u - 1) % U) * (4 * BL)  # h_{t-1} staging slot
                        wr = u * (4 * BL)              # h_t staging slot
                        wo = l * KH * G
                        # One psum BANK per j-half; all 4 K-tiles accumulate
                        # into it. Exactly one start=True per bank per step
                        # (clears the whole bank's has_written bits); every
                        # chunk's first write then overwrites (its bits are
                        # clear) and later k's accumulate — saving a second
                        # DVE add. k-pair sweeps are still emitted kappa-outer
                        # so the next step's k01 sweeps depend only on h q0/q1.
                        pst = [None, None]

                        def mm_sweep(ki, ci):
                            if ki == 0:
                                pst[ci] = psA.tile(
                                    [P, 8 * BL], f32,
                                    tag=("bkC", "bkD")[ci], name=f"ps{ci}",
                                )
                            ps = pst[ci]
                            if variant == "ewonly":
                                if ki == 0:
                                    nc.vector.memset(ps, 0.01)
                                return
                            for jj in range(2):
                                j = ci * 2 + jj
                                for tau in range(4):
                                    nch = tau * 4 + j
                                    for k in range(2 * ki, 2 * ki + 2):
                                        nc.tensor.matmul(
                                            ps[:, (tau * 2 + jj) * BL : (tau * 2 + jj + 1) * BL],
                                            lhsT=whh_sb[
                                                :,
                                                wo + k * G + nch * P : wo + k * G + (nch + 1) * P,
                                            ],
                                            rhs=stage_h[:, rd + k * BL : rd + (k + 1) * BL],
                                            start=(ki == 0 and jj == 0 and tau == 0 and k == 0),
                                            stop=(k == KH - 1),
                                            skip_group_check=True,
                                        )

                        def ew_half(ci):
                            # contiguous half block: cols [ci*64, ci*64+64),
                            # tau-major: i [0:16), f [16:32), o [32:48), g [48:64)
                            HB = 8 * BL
                            gh = g_sb[:, ci * HB : (ci + 1) * HB]
                            xh = xg_t[:, ci * HB : (ci + 1) * HB]
                            nc.vector.tensor_add(gh, pst[ci], xh)
                            nc.scalar.activation(
                                gh[:, 0 : 6 * BL], gh[:, 0 : 6 * BL], AF.Sigmoid)
                            nc.scalar.activation(
                                gh[:, 6 * BL : 8 * BL], gh[:, 6 * BL : 8 * BL], AF.Tanh)
                            ig = tmp_pool.tile([P, 2 * BL], f32, tag="ig")
                            nc.vector.tensor_mul(
                                ig, gh[:, 0 : 2 * BL], gh[:, 6 * BL : 8 * BL])
                            cq = csb[:, 2 * ci * BL : (2 * ci + 2) * BL]
                            nc.vector.tensor_mul(cq, gh[:, 2 * BL : 4 * BL], cq)
                            nc.vector.tensor_add(cq, cq, ig)
                            tc_ = tmp_pool.tile([P, 2 * BL], f32, tag="tc")
                            nc.scalar.activation(tc_, cq, AF.Tanh)
                            hdst = hscr if variant == "nohchain" else stage_h
                            hoff = (0 if variant == "nohchain" else wr) + 2 * ci * BL
                            nc.vector.tensor_mul(
                                hdst[:, hoff : hoff + 2 * BL], gh[:, 4 * BL : 6 * BL], tc_)

                        done = set()
                        emitted = set()
                        for ki, ci in SWEEP_ORDER:
                            mm_sweep(ki, ci)
                            done.add((ki, ci))
                            if variant == "mmonly":
                                continue
                            for ch in range(2):
                                if ch not in emitted and {(0, ch), (1, ch)} <= done:
                                    ew_half(ch)
                                    emitted.add(ch)
                        if variant == "mmonly":
                            for ci in range(2):
                                nc.vector.tensor_copy(
                                    stage_h[:, wr + 2 * ci * BL : wr + (2 * ci + 2) * BL],
                                    pst[ci][:, 0 : 2 * BL],
                                )
                            continue
                    # persist this body's h_t slots into hseq[it*U+1 .. it*U+U]
                    nc.sync.dma_start(
                        hseq[:, ds(it * (U * 4 * BL) + 4 * BL, U * 4 * BL)], stage_h
                    )

            # ---------- FC head ----------
            psf = psA.tile([NCLS, BL], f32, tag="bkB")
            for k in range(KH):
                nc.tensor.matmul(
                    psf,
                    lhsT=wfc_sb[:, k * NCLS : (k + 1) * NCLS],
                    rhs=hv[:, T, k * BL : (k + 1) * BL],
                    start=(k == 0), stop=(k == KH - 1),
                )
            osb = tmp_pool.tile([NCLS, BL], f32, tag="osb")
            nc.vector.tensor_scalar_add(osb, psf, bfc_sb)
            nc.sync.dma_start(out.ap().rearrange("b c -> c b"), osb)

    nc.compile()
    return nc


def build_wave(T=T_FULL, BW=40):
    """Wavefront builder: 3 layers pipelined with per-body lag of BW steps.

    Body b runs layer0 t in [b*BW,(b+1)*BW), layer1 shifted -BW, layer2 -2*BW.
    Each body starts with inter-layer projection chunks (N=BW*BL matmuls) that
    turn the previous body's h ring into the next xg ring. Each layer's
    recurrence-chain latency hides under the other layers' PE sweeps.
    """
    import concourse.bass as bass
    import concourse.mybir as mybir
    import concourse.tile as tile
    from concourse import bacc
    from concourse.bass import ds

    assert T % BW == 0
    NB = T // BW
    assert NB >= 3
    f32 = mybir.dt.float32
    bf16 = mybir.dt.bfloat16
    AF = mybir.ActivationFunctionType
    SL = 4 * BL  # 32 cols per h slot

    nc = bacc.Bacc("TRN2", target_bir_lowering=False, debug=False)

    xT = nc.dram_tensor("xT", [P, T * BL], bf16, kind="ExternalInput")
    wih0 = nc.dram_tensor("wih0", [P, G], bf16, kind="ExternalInput")
    wih12 = nc.dram_tensor("wih12", [2, KH, P, G], bf16, kind="ExternalInput")
    whh = nc.dram_tensor("whh", [L, KH, P, G], bf16, kind="ExternalInput")
    bias = nc.dram_tensor("bias", [L, P, NCH], f32, kind="ExternalInput")
    wfc = nc.dram_tensor("wfc", [KH, P, NCLS], bf16, kind="ExternalInput")
    bfc = nc.dram_tensor("bfc", [NCLS, 1], f32, kind="ExternalInput")
    out = nc.dram_tensor("out", [BL, NCLS], f32, kind="ExternalOutput")
    xg0_d = nc.dram_tensor("xg0", [P, T * P], f32, kind="Internal")

    whh_sb = nc.alloc_sbuf_tensor("whh_sb", [P, L * KH * G], bf16).ap()
    wih_sb = nc.alloc_sbuf_tensor("wih_sb", [P, 2 * KH * G], bf16).ap()
    wih0_sb = nc.alloc_sbuf_tensor("wih0_sb", [P, G], bf16).ap()
    x_sb = nc.alloc_sbuf_tensor("x_sb", [P, T * BL], bf16).ap()
    bias_sb = nc.alloc_sbuf_tensor("bias_sb", [P, L * NCH], f32).ap()
    wfc_sb = nc.alloc_sbuf_tensor("wfc_sb", [P, KH * NCLS], bf16).ap()
    bfc_sb = nc.alloc_sbuf_tensor("bfc_sb", [NCLS, 1], f32).ap()
    csb3 = nc.alloc_sbuf_tensor("csb3", [P, L * SL], f32).ap()
    ring_h = nc.alloc_sbuf_tensor("ring_h", [P, L * BW * SL], bf16).ap()
    ring_xg = nc.alloc_sbuf_tensor("ring_xg", [P, 2 * BW * P], bf16).ap()
    stage_x0 = nc.alloc_sbuf_tensor("stage_x0", [P, BW * P], f32).ap()

    CPRM = [(n % 4 // 2) * 8 + (n // 4) * 2 + (n % 4 % 2) for n in range(NCH)]
    chunks = [(t0, min(64, T - t0)) for t0 in range(0, T, 64)]
    xgv = xg0_d.ap().rearrange("p (t m) -> p t m", m=P)

    with tile.TileContext(nc) as tc:
        with (
            tc.tile_pool(name="gp", bufs=4) as g_pool,
            tc.tile_pool(name="tmpp", bufs=6) as tmp_pool,
            tc.tile_pool(name="epp", bufs=3) as ep_pool,
            tc.tile_pool(name="psA", bufs=1, space="PSUM") as psA,
            tc.tile_pool(name="psB", bufs=1, space="PSUM") as psB,
        ):
            # ---- load weights ----
            for l in range(L):
                for k in range(KH):
                    o = (l * KH + k) * G
                    nc.sync.dma_start(whh_sb[:, o : o + G], whh.ap()[l, k])
            for l in range(2):
                for k in range(KH):
                    o = (l * KH + k) * G
                    nc.sync.dma_start(wih_sb[:, o : o + G], wih12.ap()[l, k])
            nc.sync.dma_start(wih0_sb, wih0.ap())
            nc.sync.dma_start(x_sb, xT.ap())
            for l in range(L):
                nc.sync.dma_start(bias_sb[:, l * NCH : (l + 1) * NCH], bias.ap()[l])
            for k in range(KH):
                nc.sync.dma_start(wfc_sb[:, k * NCLS : (k + 1) * NCLS], wfc.ap()[k])
            nc.sync.dma_start(bfc_sb, bfc.ap())

            # ---- layer-0 input projection (full T, batched, to DRAM) ----
            for t0, tcnt in chunks:
                ncols = tcnt * BL
                for n in range(NCH):
                    ps = psB.tile([P, 512], f32, tag="pj", name="pj")
                    nc.tensor.matmul(
                        ps[:, :ncols], lhsT=wih0_sb[:, n * P : (n + 1) * P],
                        rhs=x_sb[:, t0 * BL : (t0 + tcnt) * BL],
                        start=True, stop=True,
                    )
                    ep = ep_pool.tile([P, 512], f32, tag="ep", name="ep")
                    nc.vector.tensor_scalar_add(
                        ep[:, :ncols], ps[:, :ncols], bias_sb[:, n : n + 1])
                    nc.sync.dma_start(
                        xgv[:, t0 : t0 + tcnt, CPRM[n] * BL : (CPRM[n] + 1) * BL],
                        ep[:, :ncols].rearrange("p (t b) -> p t b", b=BL),
                    )

            # ---- zero init: rings' last slot + cell states ----
            for l in range(L):
                nc.vector.memset(
                    ring_h[:, (l * BW + BW - 1) * SL : (l * BW + BW) * SL], 0.0)
            nc.vector.memset(csb3, 0.0)

            def proj_chunk(bd):
                """xg ring for boundary bd (0: l0->l1, 1: l1->l2) from h ring."""
                rh = ring_h[:, bd * BW * SL : (bd + 1) * BW * SL]
                rhv = rh.rearrange("p (t x) -> p t x", x=SL)
                rx = ring_xg[:, bd * BW * P : (bd + 1) * BW * P]
                rxv = rx.rearrange("p (t m) -> p t m", m=P)
                for n in range(NCH):
                    ps = psB.tile([P, BW * BL], f32, tag="pj", name="pjc")
                    for k in range(KH):
                        o = (bd * KH + k) * G
                        nc.tensor.matmul(
                            ps, lhsT=wih_sb[:, o + n * P : o + (n + 1) * P],
                            rhs=rhv[:, :, k * BL : (k + 1) * BL],
                            start=(k == 0), stop=(k == KH - 1),
                        )
                    nc.vector.tensor_scalar_add(
                        rxv[:, :, CPRM[n] * BL : (CPRM[n] + 1) * BL],
                        ps.rearrange("p (t b) -> p t b", b=BL),
                        bias_sb[:, (bd + 1) * NCH + n : (bd + 1) * NCH + n + 1])

            def step(l, u):
                """One wave-step of layer l at body slot u (all-static APs)."""
                if l == 0:
                    xg_t = stage_x0[:, u * P : (u + 1) * P]
                else:
                    xg_t = ring_xg[:, ((l - 1) * BW + u) * P : ((l - 1) * BW + u + 1) * P]
                rh = ring_h[:, l * BW * SL : (l + 1) * BW * SL]
                rd = ((u - 1) % BW) * SL
                wr = u * SL
                wo = l * KH * G
                g_sb = g_pool.tile([P, P], f32, tag="g", name="g_sb")
                pst = [None, None]

                def mm_sweep(ki, ci):
                    if ki == 0:
                        pst[ci] = psA.tile(
                            [P, 8 * BL], f32, tag=f"L{l}c{ci}", name=f"ps{l}{ci}")
                    ps = pst[ci]
                    for jj in range(2):
                        j = ci * 2 + jj
                        for tau in range(4):
                            nch = tau * 4 + j
                            for k in range(2 * ki, 2 * ki + 2):
                                nc.tensor.matmul(
                                    ps[:, (tau * 2 + jj) * BL : (tau * 2 + jj + 1) * BL],
                                    lhsT=whh_sb[
                                        :, wo + k * G + nch * P : wo + k * G + (nch + 1) * P],
                                    rhs=rh[:, rd + k * BL : rd + (k + 1) * BL],
                                    start=(ki == 0 and jj == 0 and tau == 0 and k == 0),
                                    stop=(k == KH - 1),
                                    skip_group_check=True,
                                )

                def ew_half(ci):
                    HB = 8 * BL
                    gh = g_sb[:, ci * HB : (ci + 1) * HB]
                    xh = xg_t[:, ci * HB : (ci + 1) * HB]
                    nc.vector.tensor_add(gh, pst[ci], xh)
                    nc.scalar.activation(
                        gh[:, 0 : 6 * BL], gh[:, 0 : 6 * BL], AF.Sigmoid)
                    nc.scalar.activation(
                        gh[:, 6 * BL : 8 * BL], gh[:, 6 * BL : 8 * BL], AF.Tanh)
                    ig = tmp_pool.tile([P, 2 * BL], f32, tag="ig", name="ig")
                    nc.vector.tensor_mul(ig, gh[:, 0 : 2 * BL], gh[:, 6 * BL : 8 * BL])
                    cq = csb3[:, l * SL + 2 * ci * BL : l * SL + (2 * ci + 2) * BL]
                    nc.vector.tensor_mul(cq, gh[:, 2 * BL : 4 * BL], cq)
                    nc.vector.tensor_add(cq, cq, ig)
                    tc_ = tmp_pool.tile([P, 2 * BL], f32, tag="tc", name="tc_")
                    nc.scalar.activation(tc_, cq, AF.Tanh)
                    nc.vector.tensor_mul(
                        rh[:, wr + 2 * ci * BL : wr + (2 * ci + 2) * BL],
                        gh[:, 4 * BL : 6 * BL], tc_)

                mm_sweep(0, 0)
                mm_sweep(0, 1)
                mm_sweep(1, 0)
                ew_half(0)
                mm_sweep(1, 1)
                ew_half(1)

            def x0_dma(off_expr):
                nc.sync.dma_start(stage_x0, xg0_d.ap()[:, off_expr])

            def body(layers, b_static=None, it=None):
                if 0 in layers:
                    if it is not None:
                        x0_dma(ds((it + 2) * (BW * P), BW * P))
                    else:
                        x0_dma(slice(b_static * BW * P, (b_static + 1) * BW * P))
                if 1 in layers:
                    proj_chunk(0)
                if 2 in layers:
                    proj_chunk(1)
                for u in range(BW):
                    for l in layers:
                        step(l, u)

            # prologue
            body([0], b_static=0)
            body([0, 1], b_static=1)
            # steady
            with tc.For_i(0, NB - 2, 1, hint_engines=(mybir.EngineType.PE,)) as it:
                body([0, 1, 2], it=it)
            # epilogue
            body([1, 2], b_static=NB)
            body([2], b_static=NB + 1)

            # ---- FC head ----
            psf = psB.tile([NCLS, BL], f32, tag="fc", name="psf")
            for k in range(KH):
                nc.tensor.matmul(
                    psf, lhsT=wfc_sb[:, k * NCLS : (k + 1) * NCLS],
                    rhs=ring_h[:, (2 * BW + BW - 1) * SL + k * BL :
                               (2 * BW + BW - 1) * SL + (k + 1) * BL],
                    start=(k == 0), stop=(k == KH - 1),
                )
            osb = tmp_pool.tile([NCLS, BL], f32, tag="osb", name="osb")
            nc.vector.tensor_scalar_add(osb, psf, bfc_sb)
            nc.sync.dma_start(out.ap().rearrange("b c -> c b"), osb)

    nc.compile()
    return nc


# ---------------- host-side input prep ----------------

_GATE_PERM = np.concatenate(
    [np.arange(0, H), np.arange(H, 2 * H), np.arange(3 * H, 4 * H), np.arange(2 * H, 3 * H)]
)  # reorder gate blocks [i, f, g, o] -> [i, f, o, g]


def prep_weights(W_ih0, W_ih_rest, W_hh, b_ih, b_hh, W_fc, b_fc, T=T_FULL):
    """Host-side: permute/transpose/tile/cast weights into kernel input layout."""
    W_ih0 = np.asarray(W_ih0, np.float32)[_GATE_PERM]          # [G, C]
    wih0 = np.zeros((P, G), np.float32)
    wih0[:C] = W_ih0.T                                          # K-padded lhsT
    wih12 = np.stack(
        [np.asarray(W_ih_rest[i], np.float32)[_GATE_PERM].T.reshape(KH, P, G) for i in range(L - 1)]
    )                                                           # [2, KH, P, G]
    whh = np.stack(
        [np.asarray(W_hh[i], np.float32)[_GATE_PERM].T.reshape(KH, P, G) for i in range(L)]
    )                                                           # [L, KH, P, G]
    bsum = (np.asarray(b_ih, np.float32) + np.asarray(b_hh, np.float32))[:, _GATE_PERM]
    bias = np.ascontiguousarray(bsum.reshape(L, NCH, P).transpose(0, 2, 1))  # [L, P, NCH]
    wfc = np.asarray(W_fc, np.float32).T.reshape(KH, P, NCLS)   # [KH, P, NCLS]
    bfc = np.asarray(b_fc, np.float32).reshape(NCLS, 1)
    return {
        "wih0": wih0.astype(BF16),
        "wih12": wih12.astype(BF16),
        "whh": whh.astype(BF16),
        "bias": np.ascontiguousarray(bias, np.float32),
        "wfc": wfc.astype(BF16),
        "bfc": bfc,
    }


def prep_x_core(x_core, T=T_FULL):
    """x_core [BL, C, T] -> padded xT [P, T*BL] bf16 (col = t*BL + b)."""
    xt = np.zeros((P, T * BL), np.float32)
    xt[:C] = np.asarray(x_core, np.float32).transpose(1, 2, 0).reshape(C, T * BL)
    return xt.astype(BF16)


_CACHE = {}


def kernel(x, W_ih0, W_ih_rest, W_hh, b_ih, b_hh, W_fc, b_fc):
    from concourse.bass_utils import run_bass_kernel_spmd

    x = np.asarray(x, np.float32)
    wts = prep_weights(W_ih0, W_ih_rest, W_hh, b_ih, b_hh, W_fc, b_fc)
    in_maps = []
    for c in range(NCORES):
        m = dict(wts)
        m["xT"] = prep_x_core(x[c * BL : (c + 1) * BL])
        in_maps.append(m)

    if "nc" not in _CACHE:
        _CACHE["nc"] = build_wave()
    res = run_bass_kernel_spmd(_CACHE["nc"], in_maps, core_ids=list(range(NCORES)))
    return np.concatenate([r["out"] for r in res.results], axis=0).astype(np.float32)


if __name__ == "__main__":
    rng = np.random.default_rng(0)
    s = 1.0 / np.sqrt(H)
    ins = dict(
        x=rng.standard_normal((B, C, T_FULL), dtype=np.float32),
        W_ih0=rng.uniform(-s, s, (G, C)).astype(np.float32),
        W_ih_rest=rng.uniform(-s, s, (L - 1, G, H)).astype(np.float32),
        W_hh=rng.uniform(-s, s, (L, G, H)).astype(np.float32),
        b_ih=rng.uniform(-s, s, (L, G)).astype(np.float32),
        b_hh=rng.uniform(-s, s, (L, G)).astype(np.float32),
        W_fc=rng.uniform(-s, s, (NCLS, H)).astype(np.float32),
        b_fc=rng.uniform(-s, s, (NCLS,)).astype(np.float32),
    )
    out = kernel(**ins)
    print(out.shape, out.dtype, np.abs(out).max())
